# revision 1
# baseline (speedup 1.0000x reference)
"""Trainium2 Bass kernel for nn_Attention_77025943487081.

Sharding: batch (4) data-parallel x 2-way head tensor-parallel over 8 cores.
Core c handles batch c//2 and heads [8*(c%2), 8*(c%2)+8). Each core produces a
partial c_proj output (contribution of its 512 input channels); the host adds
the two partials per batch plus the c_proj bias.

The per-head Conv2D projections, cross-head mixes, position projections and
biases are algebraically folded (on host) into dense matrices so the device
only runs plain matmuls:
  q[s, (g,d)] = sum_e x[s,e] * Mq[e,(g,d)] + sum_p pos[s,p] * Mq_p[p,(g,d)] + bq[(g,d)]
The 1/sqrt(D) score scale is folded into the Q-side matrices. All matmul
operands are float32r (fp32 bits, full-rate PE streaming); softmax statistics
ride along as an extra ones-column appended to V, so the PV matmul emits the
denominators in PSUM row 64 for free.
"""

import numpy as np
from contextlib import ExitStack

import concourse.bass as bass
import concourse.tile as tile
from concourse import bacc, mybir
from concourse.bass_utils import run_bass_kernel_spmd

F32 = mybir.dt.float32
F32R = mybir.dt.float32r

B, S, E, H, D, P = 4, 2048, 1024, 16, 64, 64
G = 8            # heads per core
NC = 8           # cores
EC = 9           # contraction chunks: 8 x 128 hidden + 1 (pos+bias, padded)
QKD = G * D      # 512 = per-core q (or k) width
ACT_EXP = mybir.ActivationFunctionType.Exp


def build_nc():
    nc = bacc.Bacc("TRN2", target_bir_lowering=False, debug=False, num_devices=NC)
    xT = nc.dram_tensor("xT", [EC, 128, S], F32R, kind="ExternalInput").ap()
    mqk = nc.dram_tensor("Mqk", [EC, 128, 2 * QKD], F32R, kind="ExternalInput").ap()
    mv = nc.dram_tensor("Mv", [EC, 128, QKD], F32R, kind="ExternalInput").ap()
    wc = nc.dram_tensor("Wc", [4, 128, E], F32R, kind="ExternalInput").ap()
    onesd = nc.dram_tensor("ones", [128, 128], F32R, kind="ExternalInput").ap()
    out = nc.dram_tensor("out", [S, E], F32, kind="ExternalOutput").ap()

    NT = S // 128  # 16 sequence tiles

    with nc.allow_low_precision("float32r staging of matmul operands"), \
         tile.TileContext(nc) as tc, ExitStack() as top:
        # pools alive across phases
        vaug_p = top.enter_context(tc.tile_pool(name="vaug", bufs=1))
        const_p = top.enter_context(tc.tile_pool(name="const", bufs=1))
        qk_p = top.enter_context(tc.tile_pool(name="qkt", bufs=1))

        v_aug = vaug_p.tile([128, NT, G, D + 1], F32R)  # [k-part, s-tile, head, d|ones]
        ones_col = const_p.tile([1, 64], F32R)
        # resident qT/kT: chunks 0-3 = qT head-pairs, 4-7 = kT
        qkt = [qk_p.tile([128, S], F32R, name=f"qkt{m}") for m in range(8)]
        nc.sync.dma_start(out=ones_col, in_=onesd[0:1, 0:64])
        nc.sync.dma_start(out=v_aug[:, :, :, D:D + 1], in_=onesd[:, 0:NT * G])

        # ---------------- phase 1: projections ----------------
        # ss-outer streaming of x^T slices; q/k/v psum accumulated over the 9
        # contraction chunks and copied straight into resident SBUF tiles.
        with tc.tile_pool(name="ph1", bufs=1) as ph1, \
             tc.tile_pool(name="xtss", bufs=2) as xtss_p, \
             tc.tile_pool(name="ps_qk", bufs=8, space="PSUM") as ps_qk:
            mqk_sb = ph1.tile([128, EC, 2 * QKD], F32R)
            mv_sb = ph1.tile([128, EC, QKD], F32R)
            for ec in range(EC):
                nc.sync.dma_start(out=mqk_sb[:, ec, :], in_=mqk[ec])
            for ec in range(EC):
                nc.sync.dma_start(out=mv_sb[:, ec, :], in_=mv[ec])

            for ss in range(4):
                xtss = xtss_p.tile([128, EC, 512], F32R)
                for ec in range(EC):
                    nc.sync.dma_start(out=xtss[:, ec, :],
                                      in_=xT[ec][:, ss * 512:(ss + 1) * 512])
                pss = [ps_qk.tile([128, 512], F32, tag="qk", name=f"qkps{m}")
                       for m in range(8)]
                for ec in range(EC):
                    for m in range(8):
                        nc.tensor.matmul(
                            pss[m][:, :],
                            mqk_sb[:, ec, m * 128:(m + 1) * 128],
                            xtss[:, ec, :],
                            start=(ec == 0), stop=(ec == EC - 1),
                        )
                for m in range(8):
                    nc.vector.tensor_copy(qkt[m][:, ss * 512:(ss + 1) * 512],
                                          pss[m][:, :])
                # V in natural [s, (g,d)] layout, into v_aug (col D = ones)
                for sti in range(4):
                    stt = ss * 4 + sti
                    pv = ps_qk.tile([128, 512], F32, tag="qk", name=f"vps{sti}")
                    for ec in range(EC):
                        nc.tensor.matmul(
                            pv[:, :],
                            xtss[:, ec, sti * 128:(sti + 1) * 128],
                            mv_sb[:, ec, :],
                            start=(ec == 0), stop=(ec == EC - 1),
                        )
                    nc.vector.tensor_copy(
                        v_aug[:, stt, :, 0:D],
                        pv[:, :].rearrange("p (g d) -> p g d", g=G),
                    )

        # ---------------- phase 2: attention ----------------
        with tc.tile_pool(name="oT", bufs=1) as oT_p:
            oT = oT_p.tile([128, 4, S], F32R)  # [(2 heads)*64 part, head-pair, q]
            with tc.tile_pool(name="pt", bufs=4) as pt_p, \
                 tc.tile_pool(name="rcp", bufs=4) as rcp_p, \
                 tc.tile_pool(name="ps_st", bufs=4, space="PSUM") as ps_st, \
                 tc.tile_pool(name="ps_o", bufs=1, space="PSUM") as ps_o:
                for h in range(G):
                    m, half = h // 2, h % 2
                    qt = qkt[m][64 * half:64 * half + 64, :]
                    kt = qkt[4 + m][64 * half:64 * half + 64, :]
                    po = ps_o.tile([65, S], F32)
                    for kc in range(NT):
                        q0 = kc * 128
                        ptile = pt_p.tile([128, S], F32R)
                        # scores^T + exp in <=512-wide chunks for deep pipelining
                        c0 = q0
                        while c0 < S:
                            c1 = min(S, (c0 // 512 + 1) * 512)
                            stp = ps_st.tile([128, 512], F32, tag="st")
                            nc.tensor.matmul(
                                stp[:, 0:c1 - c0],
                                kt[:, q0:q0 + 128],
                                qt[:, c0:c1],
                                start=True, stop=True,
                            )
                            nc.scalar.activation(
                                ptile[:, c0 - q0:c1 - q0],
                                stp[:, 0:c1 - c0],
                                ACT_EXP,
                            )
                            c0 = c1
                        # causal mask on the diagonal 128x128 block (separate tile so
                        # only the small diagonal PV matmul waits on it): keep q >= k
                        diag = pt_p.tile([128, 128], F32R, tag="diag")
                        nc.gpsimd.affine_select(
                            out=diag[:, :], in_=ptile[:, 0:128],
                            compare_op=mybir.AluOpType.is_ge,
                            fill=0.0, base=0, pattern=[[1, 128]], channel_multiplier=-1,
                        )
                        # PV accumulation (+ softmax denominator in row 64)
                        nc.tensor.matmul(
                            po[:, q0:q0 + 128], v_aug[:, kc, h, :], diag[:, :],
                            start=(kc == 0), stop=True,
                            skip_group_check=True,
                        )
                        for qb in range(kc // 4, 4):
                            qs = max(qb * 512, q0 + 128)
                            n = (qb + 1) * 512 - qs
                            if n <= 0:
                                continue
                            nc.tensor.matmul(
                                po[:, qs:qs + n],
                                v_aug[:, kc, h, :],
                                ptile[:, qs - q0:qs - q0 + n],
                                start=(kc == 0), stop=(kc == qb * 4 + 3),
                                skip_group_check=True,
                            )
                        # normalize each 512-q group as soon as its last k-chunk
                        # landed: oT[d, q] = po[d, q] / po[64, q]
                        if kc % 4 == 3:
                            qg = kc // 4
                            sl = slice(qg * 512, (qg + 1) * 512)
                            rcp = rcp_p.tile([1, 512], F32R)
                            nc.vector.reciprocal(rcp, po[64:65, sl])
                            bc_ps = ps_st.tile([64, 512], F32, tag="st", name=f"bcps{qg}")
                            nc.tensor.matmul(bc_ps[:, :], ones_col, rcp,
                                             start=True, stop=True)
                            bcst = rcp_p.tile([64, 512], F32R, tag="bcast",
                                              name=f"bcast{qg}")
                            nc.vector.tensor_copy(bcst[:, :], bc_ps[:, :])
                            nc.vector.tensor_mul(
                                oT[64 * half:64 * half + 64, m, sl],
                                po[0:64, sl], bcst[:, :],
                            )

            # ---------------- phase 3: partial c_proj ----------------
            with tc.tile_pool(name="wc", bufs=1) as wc_p, \
                 tc.tile_pool(name="ostage", bufs=3) as ostage_p, \
                 tc.tile_pool(name="ps_c", bufs=3, space="PSUM") as ps_c:
                wc_sb = wc_p.tile([128, 4, E], F32R)
                for gc in range(4):
                    nc.sync.dma_start(out=wc_sb[:, gc, :], in_=wc[gc])
                for stt in range(NT):
                    pc = ps_c.tile([128, E], F32)
                    for gc in range(4):
                        for ee in range(2):
                            nc.tensor.matmul(
                                pc[:, ee * 512:(ee + 1) * 512],
                                oT[:, gc, stt * 128:(stt + 1) * 128],
                                wc_sb[:, gc, ee * 512:(ee + 1) * 512],
                                start=(gc == 0), stop=(gc == 3),
                            )
                    ost = ostage_p.tile([128, E], F32)
                    nc.vector.tensor_copy(ost[:, :], pc[:, :])
                    nc.sync.dma_start(out=out[stt * 128:(stt + 1) * 128, :], in_=ost[:, :])

    nc.compile()
    return nc


def prep_core_inputs(hidden_states, position_states, Wq, bq, Wqh, bqh, Wk, bk,
                     Wkh, bkh, Wv, bv, Wvh, bvh, Wp, bp, Wpe, bpe, Wc, bc):
    """Build the per-core input maps (host-side weight folding + sharding)."""
    f32 = np.float32

    def fused(parity):
        hs = slice(G * parity, G * parity + G)
        mats = {}
        for name, (Wa, ba, Wh, bh, v) in {
            "q": (Wq, bq, Wqh[hs], bqh[hs], 0),
            "k": (Wk, bk, Wkh[hs], bkh[hs], 1),
            "v": (Wv, bv, Wvh[hs], bvh[hs], 2),
        }.items():
            mx = np.einsum("hed,ghd->hegd", Wa, Wh).reshape(E, QKD)
            mp = np.einsum("pd,g->pgd", Wp[:, v * D:(v + 1) * D], Wpe[v, 0, hs]).reshape(P, QKD)
            bias = (np.einsum("hd,ghd->gd", ba, Wh) + bh
                    + bp[v * D:(v + 1) * D][None, :] * Wpe[v, 0, hs][:, None]
                    + bpe[hs][:, None]).reshape(QKD)
            if name == "q":
                sc = 1.0 / np.sqrt(np.float32(D))
                mx, mp, bias = mx * sc, mp * sc, bias * sc
            mats[name] = (mx, mp, bias)
        def chunks(mx, mp, bias):
            w = mx.shape[1]
            m9 = np.zeros((EC, 128, w), f32)
            m9[:8] = mx.reshape(8, 128, w)
            m9[8, :P] = mp
            m9[8, P] = bias
            return m9
        mqk9 = np.concatenate([chunks(*mats["q"]), chunks(*mats["k"])], axis=2)
        mv9 = chunks(*mats["v"])
        wc4 = np.ascontiguousarray(
            Wc.reshape(H, D, E)[hs].reshape(QKD, E).reshape(4, 128, E).astype(f32))
        return np.ascontiguousarray(mqk9), np.ascontiguousarray(mv9), wc4

    per_parity = [fused(0), fused(1)]
    ones = np.ones((128, 128), f32)

    in_maps = []
    for c in range(NC):
        b, parity = c // 2, c % 2
        x9 = np.zeros((EC, 128, S), f32)
        x9[:8] = np.ascontiguousarray(hidden_states[b].T).reshape(8, 128, S)
        x9[8, :P] = position_states[b].T
        x9[8, P] = 1.0
        mqk9, mv9, wc4 = per_parity[parity]
        in_maps.append({"xT": x9, "Mqk": mqk9, "Mv": mv9, "Wc": wc4, "ones": ones})
    return in_maps


_NC_CACHE = {}


def get_nc():
    if "nc" not in _NC_CACHE:
        _NC_CACHE["nc"] = build_nc()
    return _NC_CACHE["nc"]


def kernel(**inputs):
    nc = get_nc()
    in_maps = prep_core_inputs(**inputs)
    res = run_bass_kernel_spmd(nc, in_maps, list(range(NC)))
    bc = inputs["bc"]
    outs = [res.results[2 * b]["out"] + res.results[2 * b + 1]["out"] + bc
            for b in range(B)]
    return np.stack(outs).astype(np.float32)



# revision 8
# speedup vs baseline: 1.4608x; 1.4608x over previous
"""Trainium2 Bass kernel for nn_Attention_77025943487081.

Sharding: batch (4) data-parallel x 2-way head tensor-parallel over 8 cores.
Core c handles batch c//2 and heads [8*(c%2), 8*(c%2)+8). Each core produces a
partial c_proj output (contribution of its 512 input channels); the host adds
the two partials per batch plus the c_proj bias.

The per-head Conv2D projections, cross-head mixes, position projections and
biases are algebraically folded (on host) into dense bf16 matrices so the
device only runs plain matmuls. The 1/sqrt(D) score scale is folded into the
Q-side matrices.

Device schedule (flash-style, one pass over 512-seq blocks):
  for ss in 0..3:
    project q,k,v for seq block ss (bf16, accumulated over 9 contraction
    chunks of 128); q/k land transposed [dim, seq] in SBUF, v lands
    [seq, dim] with a ones-column appended (softmax denominator rides the
    PV matmul for free as output row 64)
    for each head: causal scores^T [k,q] (bf16 matmul) -> exp (ACT, chunks
    packed into 1024-wide PSUM tiles) -> causal mask of diagonal blocks on
    GpSimd -> PV accumulate po[d|1, q] -> reciprocal of row 64 (DVE),
    partition-broadcast (GpSimd), multiply into oT [hd, q] (DVE)
    partial c_proj of the previous block and the projections of the next
    block are interleaved between attention groups as PE filler so the PE
    never waits on exp/mask latency.
"""

import numpy as np
import ml_dtypes
from contextlib import ExitStack

import concourse.bass as bass
import concourse.tile as tile
from concourse import bacc, mybir
from concourse.bass_utils import run_bass_kernel_spmd

F32 = mybir.dt.float32
BF16 = mybir.dt.bfloat16

B, S, E, H, D, P = 4, 2048, 1024, 16, 64, 64
G = 8            # heads per core
NC = 8           # cores
EC = 9           # contraction chunks: 8 x 128 hidden + 1 (pos+bias, padded)
QKD = G * D      # 512 = per-core q (or k) width
NT = S // 128    # 16 seq tiles
ACT_EXP = mybir.ActivationFunctionType.Exp


def build_nc():
    nc = bacc.Bacc("TRN2", target_bir_lowering=False, debug=False, num_devices=NC)
    xT = nc.dram_tensor("xT", [EC, 128, S], BF16, kind="ExternalInput").ap()
    mqk = nc.dram_tensor("Mqk", [EC, 128, 2 * QKD], BF16, kind="ExternalInput").ap()
    mv = nc.dram_tensor("Mv", [EC, 128, QKD], BF16, kind="ExternalInput").ap()
    wc = nc.dram_tensor("Wc", [4, 128, E], BF16, kind="ExternalInput").ap()
    out = nc.dram_tensor("out", [S, E], F32, kind="ExternalOutput").ap()

    with nc.allow_low_precision("bf16 attention datapath"), \
         tile.TileContext(nc) as tc, ExitStack() as top:
        w_p = top.enter_context(tc.tile_pool(name="weights", bufs=1))
        xt_p = top.enter_context(tc.tile_pool(name="xt", bufs=2))
        qk_p = top.enter_context(tc.tile_pool(name="qkt", bufs=1))
        va_p = top.enter_context(tc.tile_pool(name="vaug", bufs=1))
        oT_p = top.enter_context(tc.tile_pool(name="oTt", bufs=4))
        pt_p = top.enter_context(tc.tile_pool(name="ptile", bufs=3))
        dg_p = top.enter_context(tc.tile_pool(name="diag", bufs=6))
        rc_p = top.enter_context(tc.tile_pool(name="rcp", bufs=3))
        bc_p = top.enter_context(tc.tile_pool(name="bcst", bufs=3))
        ost_p = top.enter_context(tc.tile_pool(name="ost", bufs=3))

        mqk_sb = w_p.tile([128, EC, 2 * QKD], BF16)
        mv_sb = w_p.tile([128, EC, QKD], BF16)
        wc_sb = w_p.tile([128, 4, E], BF16)
        qkt = [qk_p.tile([128, S], BF16, name=f"qkt{m}") for m in range(8)]
        v_aug = va_p.tile([128, NT, G, D + 1], BF16)

        nc.vector.memset(v_aug[:, :, :, D:D + 1], 1.0)

        # startup DMA, interleaved so the first contraction chunk lands first
        xts = [None] * 4
        xts[0] = xt_p.tile([128, EC, 512], BF16, tag="xt", name="xt0")
        for ec in range(EC):
            nc.sync.dma_start(out=mqk_sb[:, ec, :], in_=mqk[ec])
            nc.sync.dma_start(out=xts[0][:, ec, :], in_=xT[ec][:, 0:512])
        for ec in range(EC):
            nc.sync.dma_start(out=mv_sb[:, ec, :], in_=mv[ec])
        for gc in range(4):
            nc.sync.dma_start(out=wc_sb[:, gc, :], in_=wc[gc])

        def qk_mm(ps, m, xt, ec):
            nc.tensor.matmul(
                ps[:, :], mqk_sb[:, ec, m * 128:(m + 1) * 128], xt[:, ec, :],
                start=(ec == 0), stop=(ec == EC - 1))

        def v_mm(ps, sti, xt, ec):
            nc.tensor.matmul(
                ps[:, :], xt[:, ec, sti * 128:(sti + 1) * 128], mv_sb[:, ec, :],
                start=(ec == 0), stop=(ec == EC - 1))

        # ---- projections for ss=0: ec-outer so compute tracks DMA arrival
        with tc.tile_pool(name="ph10", bufs=8, space="PSUM") as p8:
            qs = [p8.tile([128, 512], F32, tag="p8", name=f"p10_{m}")
                  for m in range(8)]
            for ec in range(EC):
                for m in range(8):
                    qk_mm(qs[m], m, xts[0], ec)
            for m in range(8):
                nc.vector.tensor_copy(qkt[m][:, 0:512], qs[m][:, :])
            vs = [p8.tile([128, 512], F32, tag="p8", name=f"p10v_{sti}")
                  for sti in range(4)]
            for ec in range(EC):
                for sti in range(4):
                    v_mm(vs[sti], sti, xts[0], ec)
            for sti in range(4):
                nc.vector.tensor_copy(
                    v_aug[:, sti, :, 0:D],
                    vs[sti][:, :].rearrange("p (g d) -> p g d", g=G))

        # persistent PSUM pools: 2 + 4 + 2 = 8 banks
        pp = top.enter_context(tc.tile_pool(name="pp", bufs=2, space="PSUM"))
        stp_p = top.enter_context(tc.tile_pool(name="stp", bufs=2, space="PSUM"))
        po_p = top.enter_context(tc.tile_pool(name="po", bufs=2, space="PSUM"))

        oTs = [None] * 4
        osts = {}

        def ph1_qk_wave(ss, w):
            ma, mb = 2 * w, 2 * w + 1
            pa = pp.tile([128, 512], F32, tag="pp", name=f"qk{ss}w{w}a")
            pb = pp.tile([128, 512], F32, tag="pp", name=f"qk{ss}w{w}b")
            for ec in range(EC):
                qk_mm(pa, ma, xts[ss], ec)
                qk_mm(pb, mb, xts[ss], ec)
                if ec != EC - 1:
                    yield
            sl = slice(ss * 512, ss * 512 + 512)
            nc.vector.tensor_copy(qkt[ma][:, sl], pa[:, :])
            nc.vector.tensor_copy(qkt[mb][:, sl], pb[:, :])
            yield

        def ph1_v_wave(ss, w):
            ps = []
            for sti in (2 * w, 2 * w + 1):
                pv = pp.tile([128, 512], F32, tag="pp", name=f"v{ss}s{sti}")
                for ec in range(EC):
                    v_mm(pv, sti, xts[ss], ec)
                    if ec % 3 == 2 and not (sti % 2 == 1 and ec == EC - 1):
                        yield
                ps.append((sti, pv))
            for sti, pv in ps:
                nc.vector.tensor_copy(
                    v_aug[:, 4 * ss + sti, :, 0:D],
                    pv[:, :].rearrange("p (g d) -> p g d", g=G))
            yield

        def ph3_tile(ss, qb):
            """Partial c_proj for seq tile 4*ss+qb; yields every 2 matmuls."""
            oT = oTs[ss]
            pca = pp.tile([128, 512], F32, tag="pp", name=f"pc{ss}q{qb}a")
            for hdb in range(4):
                nc.tensor.matmul(
                    pca[:, :], oT[:, hdb, qb * 128:qb * 128 + 128],
                    wc_sb[:, hdb, 0:512], start=(hdb == 0), stop=(hdb == 3))
                if hdb % 2 == 1:
                    yield
            pcb = pp.tile([128, 512], F32, tag="pp", name=f"pc{ss}q{qb}b")
            for hdb in range(4):
                nc.tensor.matmul(
                    pcb[:, :], oT[:, hdb, qb * 128:qb * 128 + 128],
                    wc_sb[:, hdb, 512:1024], start=(hdb == 0), stop=(hdb == 3))
                if hdb == 1:
                    yield
            ost = ost_p.tile([128, E], F32, tag="ost", name=f"ost{ss}q{qb}")
            nc.vector.tensor_copy(ost[:, 0:512], pca[:, :])
            nc.vector.tensor_copy(ost[:, 512:1024], pcb[:, :])
            stt = 4 * ss + qb
            nc.sync.dma_start(out=out[stt * 128:(stt + 1) * 128, :],
                              in_=ost[:, :])
            yield

        def attn_head(ss, h, pull):
            m, half = h // 2, h % 2
            qt = qkt[m][64 * half:64 * half + 64, :]
            kt = qkt[4 + m][64 * half:64 * half + 64, :]
            po = po_p.tile([65, 512], F32, tag="po", name=f"po{ss}h{h}")
            blk0 = 512 * ss

            # chunk list (kc, q_lo, width), greedy-packed into 1024-wide stp
            # tiles; a chunk may not cross a 512-col PSUM bank boundary
            chunks = []
            for kc in range(4 * ss + 4):
                qlo = max(128 * kc, blk0)
                chunks.append((kc, qlo, 512 * (ss + 1) - qlo))
            groups, cur, off = [], [], 0
            for kc, qlo, wd in chunks:
                if cur and (off + wd > 1024 or off // 512 != (off + wd - 1) // 512):
                    groups.append(cur)
                    cur, off = [], 0
                cur.append((kc, qlo, wd, off))
                off += wd
            groups.append(cur)

            state = {"first_pv": True}

            def emit_scores(g):
                stp = stp_p.tile([128, 1024], F32, tag="stp", name=f"stp{ss}h{h}")
                ptile = pt_p.tile([128, 1024], BF16, tag="pt", name=f"pt{ss}h{h}")
                for kc, qlo, wd, off in g:
                    nc.tensor.matmul(
                        stp[:, off:off + wd],
                        kt[:, 128 * kc:128 * kc + 128],
                        qt[:, qlo:qlo + wd],
                        start=True, stop=True)
                tot = g[-1][3] + g[-1][2]
                nc.scalar.activation(ptile[:, 0:tot], stp[:, 0:tot], ACT_EXP)
                # diagonal-block causal masks (keep q >= k), off the PE path
                dgs = {}
                for kc, qlo, wd, off in g:
                    if kc >= 4 * ss:
                        dg = dg_p.tile([128, 128], BF16, tag="dg",
                                       name=f"dg{ss}h{h}")
                        nc.gpsimd.affine_select(
                            out=dg[:, :], in_=ptile[:, off:off + 128],
                            compare_op=mybir.AluOpType.is_ge,
                            fill=0.0, base=0, pattern=[[1, 128]],
                            channel_multiplier=-1)
                        dgs[kc] = dg
                return ptile, dgs

            def pv_mm(kc, rhs, col0, ncol):
                nc.tensor.matmul(
                    po[:, col0:col0 + ncol], v_aug[:, kc, h, :], rhs,
                    start=state["first_pv"], stop=False,
                    skip_group_check=True)
                state["first_pv"] = False

            def emit_pv(g, ptile, dgs):
                # non-diagonal parts first (they only wait on exp), masked
                # diagonal blocks last (they also wait on the gpsimd mask)
                for kc, qlo, wd, off in g:
                    if kc >= 4 * ss and wd > 128:
                        pv_mm(kc, ptile[:, off + 128:off + wd],
                              qlo + 128 - blk0, wd - 128)
                    elif kc < 4 * ss:
                        pv_mm(kc, ptile[:, off:off + wd], qlo - blk0, wd)
                for kc, qlo, wd, off in g:
                    if kc >= 4 * ss:
                        pv_mm(kc, dgs[kc][:, :], qlo - blk0, 128)

            prev = None
            for g in groups:
                sc = emit_scores(g)
                pull()
                if prev is not None:
                    emit_pv(*prev)
                    pull()
                prev = (g, *sc)
            emit_pv(*prev)
            pull()

            # normalize: oT[hd, q] = po[d, q] * (1 / po[64, q])
            rcp = rc_p.tile([1, 512], F32, tag="rc", name=f"rcp{ss}h{h}")
            nc.vector.reciprocal(rcp, po[64:65, :])
            bcst = bc_p.tile([64, 512], F32, tag="bc", name=f"bc{ss}h{h}")
            nc.gpsimd.partition_broadcast(bcst[:, :], rcp[:, :])
            nc.vector.tensor_mul(
                oTs[ss][64 * half:64 * half + 64, m, :],
                po[0:64, :], bcst[:, :])

        # ---- main sweep over 512-seq blocks
        for ss in range(4):
            if ss < 3:
                xts[ss + 1] = xt_p.tile([128, EC, 512], BF16, tag="xt",
                                        name=f"xt{ss + 1}")
                for ec in range(EC):
                    nc.sync.dma_start(
                        out=xts[ss + 1][:, ec, :],
                        in_=xT[ec][:, (ss + 1) * 512:(ss + 2) * 512])
            oTs[ss] = oT_p.tile([128, 4, 512], BF16, tag="oT", name=f"oT{ss}")

            # PE filler work pulled between attention groups: blocks 0-2
            # get the next block's projections, the last block gets all the
            # deferred c_proj tiles (it has no projections left to run)
            gen_list = []
            if ss < 3:
                for w in range(4):
                    gen_list.append(ph1_qk_wave(ss + 1, w))
                for w in range(2):
                    gen_list.append(ph1_v_wave(ss + 1, w))
                n_steps_total = 54
            else:
                for pss in range(3):
                    for qb in range(4):
                        gen_list.append(ph3_tile(pss, qb))
                n_steps_total = 60

            gen_iter = iter(gen_list)
            current = {"g": None}

            def pull_one():
                while True:
                    if current["g"] is None:
                        current["g"] = next(gen_iter, None)
                        if current["g"] is None:
                            return False
                    try:
                        next(current["g"])
                        return True
                    except StopIteration:
                        current["g"] = None

            # count groups per head (identical for every head at this ss)
            _g, _off = 1, 0
            for kc in range(4 * ss + 4):
                wd = 512 * (ss + 1) - max(128 * kc, 512 * ss)
                if _off and (_off + wd > 1024 or _off // 512 != (_off + wd - 1) // 512):
                    _g += 1
                    _off = 0
                _off += wd
            head_lo = 1 if ss in (1, 2) else 0
            total_groups = (8 - head_lo) * _g

            pull_count = {"n": 0, "done": 0}

            def make_pull(active):
                def pull():
                    if not active:
                        return
                    pull_count["n"] += 1
                    target = (n_steps_total * pull_count["n"] + total_groups - 1) \
                        // max(total_groups, 1)
                    while pull_count["done"] < target:
                        if not pull_one():
                            return
                        pull_count["done"] += 1
                return pull

            for h in range(G):
                attn_head(ss, h, make_pull(h >= head_lo))
            # drain any remaining filler steps
            while pull_one():
                pass

        # final block's c_proj
        for qb in range(4):
            for _ in ph3_tile(3, qb):
                pass

    nc.compile()
    return nc


def prep_core_inputs(hidden_states, position_states, Wq, bq, Wqh, bqh, Wk, bk,
                     Wkh, bkh, Wv, bv, Wvh, bvh, Wp, bp, Wpe, bpe, Wc, bc):
    """Build the per-core input maps (host-side weight folding + sharding)."""
    bf16 = ml_dtypes.bfloat16
    f32 = np.float32

    def fused(parity):
        hs = slice(G * parity, G * parity + G)
        mats = {}
        for name, (Wa, ba, Wh, bh, v) in {
            "q": (Wq, bq, Wqh[hs], bqh[hs], 0),
            "k": (Wk, bk, Wkh[hs], bkh[hs], 1),
            "v": (Wv, bv, Wvh[hs], bvh[hs], 2),
        }.items():
            mx = np.einsum("hed,ghd->hegd", Wa, Wh).reshape(E, QKD)
            mp = np.einsum("pd,g->pgd", Wp[:, v * D:(v + 1) * D], Wpe[v, 0, hs]).reshape(P, QKD)
            bias = (np.einsum("hd,ghd->gd", ba, Wh) + bh
                    + bp[v * D:(v + 1) * D][None, :] * Wpe[v, 0, hs][:, None]
                    + bpe[hs][:, None]).reshape(QKD)
            if name == "q":
                sc = 1.0 / np.sqrt(np.float32(D))
                mx, mp, bias = mx * sc, mp * sc, bias * sc
            mats[name] = (mx, mp, bias)

        def chunks(mx, mp, bias):
            w = mx.shape[1]
            m9 = np.zeros((EC, 128, w), f32)
            m9[:8] = mx.reshape(8, 128, w)
            m9[8, :P] = mp
            m9[8, P] = bias
            return m9
        mqk9 = np.concatenate([chunks(*mats["q"]), chunks(*mats["k"])], axis=2)
        mv9 = chunks(*mats["v"])
        wc4 = Wc.reshape(H, D, E)[hs].reshape(QKD, E).reshape(4, 128, E)
        return (np.ascontiguousarray(mqk9).astype(bf16),
                np.ascontiguousarray(mv9).astype(bf16),
                np.ascontiguousarray(wc4).astype(bf16))

    per_parity = [fused(0), fused(1)]

    in_maps = []
    for c in range(NC):
        b, parity = c // 2, c % 2
        x9 = np.zeros((EC, 128, S), f32)
        x9[:8] = np.ascontiguousarray(hidden_states[b].T).reshape(8, 128, S)
        x9[8, :P] = position_states[b].T
        x9[8, P] = 1.0
        mqk9, mv9, wc4 = per_parity[parity]
        in_maps.append({"xT": x9.astype(bf16), "Mqk": mqk9, "Mv": mv9,
                        "Wc": wc4})
    return in_maps


_NC_CACHE = {}


def get_nc():
    if "nc" not in _NC_CACHE:
        _NC_CACHE["nc"] = build_nc()
    return _NC_CACHE["nc"]


def kernel(**inputs):
    nc = get_nc()
    in_maps = prep_core_inputs(**inputs)
    res = run_bass_kernel_spmd(nc, in_maps, list(range(NC)))
    bc = inputs["bc"]
    outs = [res.results[2 * b]["out"] + res.results[2 * b + 1]["out"] + bc
            for b in range(B)]
    return np.stack(outs).astype(np.float32)


# revision 15
# speedup vs baseline: 1.6290x; 1.1151x over previous
"""Trainium2 Bass kernel for nn_Attention_77025943487081.

Sharding: batch (4) data-parallel x 2-way head tensor-parallel over 8 cores.
Core c handles batch c//2 and heads [8*(c%2), 8*(c%2)+8). Each core produces a
partial c_proj output (contribution of its 512 input channels); the host adds
the two partials per batch plus the c_proj bias.

The per-head Conv2D projections, cross-head mixes, position projections and
biases are algebraically folded (on host) into dense bf16 matrices so the
device only runs plain matmuls. The 1/sqrt(D) score scale is folded into the
Q-side matrices.

Device schedule (flash-style, one pass over 512-seq blocks):
  for ss in 0..3:
    project q,k,v for seq block ss (bf16, accumulated over 9 contraction
    chunks of 128); q/k land transposed [dim, seq] in SBUF, v lands
    [seq, dim] with a ones-column appended (softmax denominator rides the
    PV matmul for free as output row 64)
    for each head: causal scores^T [k,q] (bf16 matmul) -> exp (ACT, chunks
    packed into 1024-wide PSUM tiles) -> causal mask of diagonal blocks on
    GpSimd -> PV accumulate po[d|1, q] -> reciprocal of row 64 (DVE),
    partition-broadcast (GpSimd), multiply into oT [hd, q] (DVE)
    partial c_proj of the previous block and the projections of the next
    block are interleaved between attention groups as PE filler so the PE
    never waits on exp/mask latency.
"""

import numpy as np
import ml_dtypes
from contextlib import ExitStack

import concourse.bass as bass
import concourse.tile as tile
from concourse import bacc, mybir
from concourse.bass_utils import run_bass_kernel_spmd

F32 = mybir.dt.float32
BF16 = mybir.dt.bfloat16
FP8 = mybir.dt.float8e4
EXP_SHIFT = -5.0  # exp(s-5): keeps fp8e4m3 probabilities in range

B, S, E, H, D, P = 4, 2048, 1024, 16, 64, 64
G = 8            # heads per core
NC = 8           # cores
EC = 9           # contraction chunks: 8 x 128 hidden + 1 (pos+bias, padded)
QKD = G * D      # 512 = per-core q (or k) width
NT = S // 128    # 16 seq tiles
ACT_EXP = mybir.ActivationFunctionType.Exp


def build_nc():
    nc = bacc.Bacc("TRN2", target_bir_lowering=False, debug=False, num_devices=NC)
    xT = nc.dram_tensor("xT", [EC, 128, S], BF16, kind="ExternalInput").ap()
    mqk = nc.dram_tensor("Mqk", [EC, 128, 2 * QKD], BF16, kind="ExternalInput").ap()
    mv = nc.dram_tensor("Mv", [EC, 128, QKD], BF16, kind="ExternalInput").ap()
    wc = nc.dram_tensor("Wc", [4, 128, E], BF16, kind="ExternalInput").ap()
    out = nc.dram_tensor("out", [S, E], F32, kind="ExternalOutput").ap()

    with nc.allow_low_precision("bf16 attention datapath"), \
         tile.TileContext(nc) as tc, ExitStack() as top:
        w_p = top.enter_context(tc.tile_pool(name="weights", bufs=1))
        xt_p = top.enter_context(tc.tile_pool(name="xt", bufs=2))
        qk_p = top.enter_context(tc.tile_pool(name="qkt", bufs=1))
        va_p = top.enter_context(tc.tile_pool(name="vaug", bufs=1))
        oT_p = top.enter_context(tc.tile_pool(name="oTt", bufs=4))
        pt_p = top.enter_context(tc.tile_pool(name="ptile", bufs=3))
        dg_p = top.enter_context(tc.tile_pool(name="diag", bufs=6))
        rc_p = top.enter_context(tc.tile_pool(name="rcp", bufs=3))
        bc_p = top.enter_context(tc.tile_pool(name="bcst", bufs=3))
        ost_p = top.enter_context(tc.tile_pool(name="ost", bufs=3))

        mqk_sb = w_p.tile([128, EC, 2 * QKD], BF16)
        mv_sb = w_p.tile([128, EC, QKD], BF16)
        wc_sb = w_p.tile([128, 4, E], BF16)
        qkt = [qk_p.tile([128, S], BF16, name=f"qkt{m}") for m in range(8)]
        v_aug = va_p.tile([128, NT, G, D + 1], BF16)
        # fp8 copy of v in (k-tile-pair, parity) layout for DoubleRow PV
        v8 = va_p.tile([128, NT // 2, G, 2, 80], FP8)

        eshift = w_p.tile([128, 1], F32)
        nc.vector.memset(v_aug[:, :, :, D:D + 1], 1.0)
        nc.vector.memset(v8[:, :, :, :, D:D + 1], 1.0)
        nc.vector.memset(eshift[:, :], EXP_SHIFT)

        # startup DMA, interleaved so the first contraction chunk lands first
        xts = [None] * 4
        xts[0] = xt_p.tile([128, EC, 512], BF16, tag="xt", name="xt0")
        for ec in range(EC):
            nc.sync.dma_start(out=mqk_sb[:, ec, :], in_=mqk[ec])
            nc.sync.dma_start(out=xts[0][:, ec, :], in_=xT[ec][:, 0:512])
        for ec in range(EC):
            nc.sync.dma_start(out=mv_sb[:, ec, :], in_=mv[ec])
        for gc in range(4):
            nc.sync.dma_start(out=wc_sb[:, gc, :], in_=wc[gc])

        def qk_mm(ps, m, xt, ec):
            nc.tensor.matmul(
                ps[:, :], mqk_sb[:, ec, m * 128:(m + 1) * 128], xt[:, ec, :],
                start=(ec == 0), stop=(ec == EC - 1))

        def v_mm(ps, sti, xt, ec):
            nc.tensor.matmul(
                ps[:, :], xt[:, ec, sti * 128:(sti + 1) * 128], mv_sb[:, ec, :],
                start=(ec == 0), stop=(ec == EC - 1))

        # persistent PSUM pools: 2 + 4 + 2 = 8 banks
        pp = top.enter_context(tc.tile_pool(name="pp", bufs=2, space="PSUM"))
        stp_p = top.enter_context(tc.tile_pool(name="stp", bufs=2, space="PSUM"))
        po_p = top.enter_context(tc.tile_pool(name="po", bufs=2, space="PSUM"))

        oTs = [None] * 4
        osts = {}

        def ph1_qk_wave(ss, w):
            ma, mb = w, 4 + w
            pa = pp.tile([128, 512], F32, tag="pp", name=f"qk{ss}w{w}a")
            pb = pp.tile([128, 512], F32, tag="pp", name=f"qk{ss}w{w}b")
            for ec in range(EC):
                qk_mm(pa, ma, xts[ss], ec)
                qk_mm(pb, mb, xts[ss], ec)
                if ec != EC - 1:
                    yield
            sl = slice(ss * 512, ss * 512 + 512)
            nc.vector.tensor_copy(qkt[ma][:, sl], pa[:, :])
            nc.vector.tensor_copy(qkt[mb][:, sl], pb[:, :])
            yield

        def ph1_v_wave(ss, w):
            for sti in (2 * w, 2 * w + 1):
                pv = pp.tile([128, 512], F32, tag="pp", name=f"v{ss}s{sti}")
                for ec in range(EC):
                    v_mm(pv, sti, xts[ss], ec)
                    if ec % 3 == 2 and ec != EC - 1:
                        yield
                stt = 4 * ss + sti
                nc.vector.tensor_copy(
                    v_aug[:, stt, :, 0:D],
                    pv[:, :].rearrange("p (g d) -> p g d", g=G))
                nc.scalar.activation(
                    v8[:, stt // 2, :, stt % 2, 0:D],
                    pv[:, :].rearrange("p (g d) -> p g d", g=G),
                    mybir.ActivationFunctionType.Copy)
                yield

        def ph3_tile(ss, qb):
            """Partial c_proj for seq tile 4*ss+qb; yields every 2 matmuls."""
            oT = oTs[ss]
            pca = pp.tile([128, 512], F32, tag="pp", name=f"pc{ss}q{qb}a")
            for hdb in range(4):
                nc.tensor.matmul(
                    pca[:, :], oT[:, hdb, qb * 128:qb * 128 + 128],
                    wc_sb[:, hdb, 0:512], start=(hdb == 0), stop=(hdb == 3))
                if hdb % 2 == 1:
                    yield
            stt = 4 * ss + qb
            ost = ost_p.tile([128, E], F32, tag="ost", name=f"ost{ss}q{qb}")
            nc.vector.tensor_copy(ost[:, 0:512], pca[:, :])
            nc.sync.dma_start(out=out[stt * 128:(stt + 1) * 128, 0:512],
                              in_=ost[:, 0:512])
            pcb = pp.tile([128, 512], F32, tag="pp", name=f"pc{ss}q{qb}b")
            for hdb in range(4):
                nc.tensor.matmul(
                    pcb[:, :], oT[:, hdb, qb * 128:qb * 128 + 128],
                    wc_sb[:, hdb, 512:1024], start=(hdb == 0), stop=(hdb == 3))
                if hdb == 1:
                    yield
            nc.vector.tensor_copy(ost[:, 512:1024], pcb[:, :])
            nc.sync.dma_start(out=out[stt * 128:(stt + 1) * 128, 512:1024],
                              in_=ost[:, 512:1024])
            yield

        def attn_head(ss, h, pull):
            m, half = h // 2, h % 2
            qt = qkt[m][64 * half:64 * half + 64, :]
            kt = qkt[4 + m][64 * half:64 * half + 64, :]
            po = po_p.tile([65, 512], F32, tag="po", name=f"po{ss}h{h}")
            blk0 = 512 * ss

            # units: fp8-DoubleRow pairs of full 512-wide k-tiles
            # (kc < 4*ss), then the 4 diagonal k-tiles in bf16, greedy-packed
            # into 1024-wide stp tiles (no chunk crosses a PSUM bank)
            chunks = []
            for kc in range(4 * ss, 4 * ss + 4):
                qlo = 128 * kc
                chunks.append((kc, qlo, 512 * (ss + 1) - qlo))
            groups, cur, off = [], [], 0
            for kc, qlo, wd in chunks:
                if cur and (off + wd > 1024 or off // 512 != (off + wd - 1) // 512):
                    groups.append(cur)
                    cur, off = [], 0
                cur.append((kc, qlo, wd, off))
                off += wd
            groups.append(cur)

            state = {"first_pv": True}

            def emit_scores_pair(j):
                stp = stp_p.tile([128, 1024], F32, tag="stp", name=f"stp{ss}h{h}")
                pt8 = pt_p.tile([128, 2, 512], FP8, tag="pt8", name=f"p8{ss}h{h}")
                for par in range(2):
                    kc = 2 * j + par
                    nc.tensor.matmul(
                        stp[:, 512 * par:512 * par + 512],
                        kt[:, 128 * kc:128 * kc + 128],
                        qt[:, blk0:blk0 + 512],
                        start=True, stop=True)
                nc.scalar.activation(
                    pt8[:, :, :].rearrange("p a b -> p (a b)"),
                    stp[:, 0:1024], ACT_EXP, bias=eshift[:, :])
                return pt8

            def emit_pv_pair(j, pt8):
                nc.tensor.matmul(
                    po[:, 0:512], v8[:, j, h, :, 0:D + 1], pt8[:, :, :],
                    start=state["first_pv"], stop=False,
                    perf_mode=mybir.MatmulPerfMode.DoubleRow,
                    skip_group_check=True)
                state["first_pv"] = False

            def emit_scores(g):
                stp = stp_p.tile([128, 1024], F32, tag="stp", name=f"stp{ss}h{h}")
                ptile = pt_p.tile([128, 1024], BF16, tag="pt", name=f"pt{ss}h{h}")
                for kc, qlo, wd, off in g:
                    nc.tensor.matmul(
                        stp[:, off:off + wd],
                        kt[:, 128 * kc:128 * kc + 128],
                        qt[:, qlo:qlo + wd],
                        start=True, stop=True)
                tot = g[-1][3] + g[-1][2]
                nc.scalar.activation(ptile[:, 0:tot], stp[:, 0:tot], ACT_EXP,
                                     bias=eshift[:, :])
                # diagonal-block causal masks (keep q >= k), off the PE path
                dgs = {}
                for kc, qlo, wd, off in g:
                    dg = dg_p.tile([128, 128], BF16, tag="dg",
                                   name=f"dg{ss}h{h}")
                    nc.gpsimd.affine_select(
                        out=dg[:, :], in_=ptile[:, off:off + 128],
                        compare_op=mybir.AluOpType.is_ge,
                        fill=0.0, base=0, pattern=[[1, 128]],
                        channel_multiplier=-1)
                    dgs[kc] = dg
                return ptile, dgs

            def pv_mm(kc, rhs, col0, ncol):
                nc.tensor.matmul(
                    po[:, col0:col0 + ncol], v_aug[:, kc, h, :], rhs,
                    start=state["first_pv"], stop=False,
                    skip_group_check=True)
                state["first_pv"] = False

            def emit_pv(g, ptile, dgs):
                # off-diagonal parts first (they only wait on exp), masked
                # diagonal blocks last (they also wait on the gpsimd mask)
                for kc, qlo, wd, off in g:
                    if wd > 128:
                        pv_mm(kc, ptile[:, off + 128:off + wd],
                              qlo + 128 - blk0, wd - 128)
                for kc, qlo, wd, off in g:
                    pv_mm(kc, dgs[kc][:, :], qlo - blk0, 128)

            units = [("pair", j) for j in range(2 * ss)] +                     [("grp", g) for g in groups]
            prev = None
            for kind, u in units:
                if kind == "pair":
                    sc = ("pair", emit_scores_pair(u), u)
                else:
                    sc = ("grp", emit_scores(u), u)
                if prev is not None:
                    if prev[0] == "pair":
                        emit_pv_pair(prev[2], prev[1])
                    else:
                        emit_pv(prev[2], *prev[1])
                    pull()
                prev = sc
            if prev[0] == "pair":
                emit_pv_pair(prev[2], prev[1])
            else:
                emit_pv(prev[2], *prev[1])
            pull()

            # normalize: oT[hd, q] = po[d, q] * (1 / po[64, q])
            rcp = rc_p.tile([1, 512], F32, tag="rc", name=f"rcp{ss}h{h}")
            nc.vector.reciprocal(rcp, po[64:65, :])
            bcst = bc_p.tile([64, 512], F32, tag="bc", name=f"bc{ss}h{h}")
            nc.gpsimd.partition_broadcast(bcst[:, :], rcp[:, :])
            nc.vector.tensor_mul(
                oTs[ss][64 * half:64 * half + 64, m, :],
                po[0:64, :], bcst[:, :])

        # ---- projections for seq block 0 (same waves, DMA-paced)
        for w in range(4):
            for _ in ph1_qk_wave(0, w):
                pass
        for w in range(2):
            for _ in ph1_v_wave(0, w):
                pass

        # ---- main sweep over 512-seq blocks
        for ss in range(4):
            if ss < 3:
                xts[ss + 1] = xt_p.tile([128, EC, 512], BF16, tag="xt",
                                        name=f"xt{ss + 1}")
                for ec in range(EC):
                    nc.sync.dma_start(
                        out=xts[ss + 1][:, ec, :],
                        in_=xT[ec][:, (ss + 1) * 512:(ss + 2) * 512])
            oTs[ss] = oT_p.tile([128, 4, 512], BF16, tag="oT", name=f"oT{ss}")

            # PE filler work pulled between attention groups: blocks 0-2
            # get the next block's projections, the last block gets all the
            # deferred c_proj tiles (it has no projections left to run)
            gen_list = []
            if ss < 3:
                for w in range(4):
                    gen_list.append(ph1_qk_wave(ss + 1, w))
                for w in range(2):
                    gen_list.append(ph1_v_wave(ss + 1, w))
                n_steps_total = 54
            else:
                for pss in range(3):
                    for qb in range(4):
                        gen_list.append(ph3_tile(pss, qb))
                n_steps_total = 60

            gen_iter = iter(gen_list)
            current = {"g": None}

            def pull_one():
                while True:
                    if current["g"] is None:
                        current["g"] = next(gen_iter, None)
                        if current["g"] is None:
                            return False
                    try:
                        next(current["g"])
                        return True
                    except StopIteration:
                        current["g"] = None

            # units per head: 2*ss fp8 pairs + 2 bf16 diagonal groups
            _g = 2 * ss + 2
            head_lo = 1 if ss in (1, 2) else 0
            total_groups = (8 - head_lo) * _g

            pull_count = {"n": 0, "done": 0}

            def make_pull(active):
                def pull():
                    if not active:
                        return
                    pull_count["n"] += 1
                    target = (n_steps_total * pull_count["n"] + total_groups - 1) \
                        // max(total_groups, 1)
                    while pull_count["done"] < target:
                        if not pull_one():
                            return
                        pull_count["done"] += 1
                return pull

            for h in range(G):
                attn_head(ss, h, make_pull(h >= head_lo))
            # drain any remaining filler steps
            while pull_one():
                pass

        # final block's c_proj
        for qb in range(4):
            for _ in ph3_tile(3, qb):
                pass

    nc.compile()
    return nc


def prep_core_inputs(hidden_states, position_states, Wq, bq, Wqh, bqh, Wk, bk,
                     Wkh, bkh, Wv, bv, Wvh, bvh, Wp, bp, Wpe, bpe, Wc, bc):
    """Build the per-core input maps (host-side weight folding + sharding)."""
    bf16 = ml_dtypes.bfloat16
    f32 = np.float32

    def fused(parity):
        hs = slice(G * parity, G * parity + G)
        mats = {}
        for name, (Wa, ba, Wh, bh, v) in {
            "q": (Wq, bq, Wqh[hs], bqh[hs], 0),
            "k": (Wk, bk, Wkh[hs], bkh[hs], 1),
            "v": (Wv, bv, Wvh[hs], bvh[hs], 2),
        }.items():
            mx = np.einsum("hed,ghd->hegd", Wa, Wh).reshape(E, QKD)
            mp = np.einsum("pd,g->pgd", Wp[:, v * D:(v + 1) * D], Wpe[v, 0, hs]).reshape(P, QKD)
            bias = (np.einsum("hd,ghd->gd", ba, Wh) + bh
                    + bp[v * D:(v + 1) * D][None, :] * Wpe[v, 0, hs][:, None]
                    + bpe[hs][:, None]).reshape(QKD)
            if name == "q":
                sc = 1.0 / np.sqrt(np.float32(D))
                mx, mp, bias = mx * sc, mp * sc, bias * sc
            mats[name] = (mx, mp, bias)

        def chunks(mx, mp, bias):
            w = mx.shape[1]
            m9 = np.zeros((EC, 128, w), f32)
            m9[:8] = mx.reshape(8, 128, w)
            m9[8, :P] = mp
            m9[8, P] = bias
            return m9
        mqk9 = np.concatenate([chunks(*mats["q"]), chunks(*mats["k"])], axis=2)
        mv9 = chunks(*mats["v"])
        wc4 = Wc.reshape(H, D, E)[hs].reshape(QKD, E).reshape(4, 128, E)
        return (np.ascontiguousarray(mqk9).astype(bf16),
                np.ascontiguousarray(mv9).astype(bf16),
                np.ascontiguousarray(wc4).astype(bf16))

    per_parity = [fused(0), fused(1)]

    in_maps = []
    for c in range(NC):
        b, parity = c // 2, c % 2
        x9 = np.zeros((EC, 128, S), f32)
        x9[:8] = np.ascontiguousarray(hidden_states[b].T).reshape(8, 128, S)
        x9[8, :P] = position_states[b].T
        x9[8, P] = 1.0
        mqk9, mv9, wc4 = per_parity[parity]
        in_maps.append({"xT": x9.astype(bf16), "Mqk": mqk9, "Mv": mv9,
                        "Wc": wc4})
    return in_maps


_NC_CACHE = {}


def get_nc():
    if "nc" not in _NC_CACHE:
        _NC_CACHE["nc"] = build_nc()
    return _NC_CACHE["nc"]


def kernel(**inputs):
    nc = get_nc()
    in_maps = prep_core_inputs(**inputs)
    res = run_bass_kernel_spmd(nc, in_maps, list(range(NC)))
    bc = inputs["bc"]
    outs = [res.results[2 * b]["out"] + res.results[2 * b + 1]["out"] + bc
            for b in range(B)]
    return np.stack(outs).astype(np.float32)
